# revision 48
# baseline (speedup 1.0000x reference)
"""VQ codebook encoding kernel for Trainium2 (8 NeuronCores, SPMD).

Problem: nn_Encoding-style soft-assignment codebook encoding.
  x: (16, 512, 64, 64) f32, codewords: (32, 512) f32, scale: (32,) f32
  logits[b,n,k] = scale[k] * (||x_bn||^2 - 2 x_bn.c_k + ||c_k||^2)
  A = softmax_k(logits);  out[b,k,c] = sum_n A (x_bn - c_k)   -> (16, 32, 512)

Sharding: data-parallel over batch B=16 -> 2 batches per core, no collectives.

Per-core dataflow (matmul operands bf16, accumulation/softmax f32):
  - x shard is cast to bf16 on host; loaded twice from HBM: natural layout
    [c,n] (contiguous per c-chunk) and transposed [n,c] via the xbar
    DMA-transpose path (one DMA per 4 n-chunks; 3D out AP folds the mid-dim
    into logical partitions in natural chunk order).
  - phase 1 (PE): S'[k,n] = sum_c W1[c,k] x[c,n] in PSUM, where
    W1 = -2*scale_k*cw[k,c]; exp on ACT with per-partition bias
    s_k*c2_k + ds_k*X2C (ds = scale - scale.max()) -> e'; PE-transpose
    e' -> [n-part, k] PSUM.
  - x2[n] = sum_c xT^2 via fused square+row-sum ops split across DVE
    (scalar_tensor_tensor accum_out) and ACT (Square accum_out).
  - softmax shift is exact for any shared per-n shift: the x2 term is applied
    AFTER the transpose as e = e' * exp(ds_k * (x2[n]-X2C)) (one broadcast
    tensor_mul + one ACT exp + one tensor_mul); ds<=0 and x2>X2C keep the
    factor in (0,1], and e' flushes only where the true weight is negligible.
  - Z = sum_k e (DVE row-reduce), reciprocal, normalize -> A (bf16).
  - phase 2 (PE): enc1[k,c] = sum_n A^T xT, asum[k] = sum_n A via ones
    column; out = enc1 - asum*cw fused on DVE (scalar_tensor_tensor); DMA out.
  - Loop fission: both batches' phase-1 emitted before phase-2s so the
    in-order PE stream stays fed during the DVE/ACT normalize chains.
"""

import numpy as np
import ml_dtypes

B, C, H, W = 16, 512, 64, 64
K = 32
N = H * W            # 4096 spatial positions
NCORES = 8
BPC = B // NCORES    # batches per core
CC = C // 128        # c chunks (4)
NSLICES = N // 512   # 8 matmul slices per batch
NCHUNKS = N // 128   # 32 n-chunks per batch
X2C = 256.0          # x2 recentering: ds<=0 and x2-256>0 keep exp(ds*(x2-X2C)) in (0,1]

_cache = {}


def _build_nc():
    import concourse.bass as bass
    import concourse.bacc as bacc
    import concourse.tile as tile
    from concourse import mybir

    f32 = mybir.dt.float32
    bf16 = mybir.dt.bfloat16
    AF = mybir.ActivationFunctionType
    ALU = mybir.AluOpType

    # Bacc (not plain Bass): its compile pipeline splits semaphore waits to
    # the 1-per-instruction hardware limit and codegens ISA subclasses —
    # required for this walrus build to accept the NEFF.
    nc = bacc.Bacc("TRN2", target_bir_lowering=False, debug=False)

    xn_d = nc.declare_dram_parameter("xn", [BPC, C, N], bf16, isOutput=False)
    cb32_d = nc.declare_dram_parameter("cblob32", [128, 577], f32, isOutput=False)
    cb16_d = nc.declare_dram_parameter("cblob16", [128, 161], bf16, isOutput=False)
    enc_d = nc.declare_dram_parameter("enc", [BPC, K, C], f32, isOutput=True)

    with tile.TileContext(nc) as tc:
        with (
            tc.tile_pool(name="consts", bufs=1) as consts,
            tc.tile_pool(name="xn", bufs=2) as xn_pool,
            tc.tile_pool(name="xt", bufs=2) as xt_pool,
            tc.tile_pool(name="sq", bufs=2) as sq_pool,
            tc.tile_pool(name="x2m", bufs=2) as x2m_pool,
            tc.tile_pool(name="fmat", bufs=2) as f_pool,
            tc.tile_pool(name="e", bufs=3) as e_pool,
            tc.tile_pool(name="eall", bufs=2) as eall_pool,
            tc.tile_pool(name="z", bufs=2) as z_pool,
            tc.tile_pool(name="a", bufs=2) as a_pool,
            tc.tile_pool(name="encsb", bufs=2) as enc_sb_pool,
            tc.tile_pool(name="nasum", bufs=2) as nasum_pool,
            tc.tile_pool(name="ps_s", bufs=2, space="PSUM") as ps_s,
            tc.tile_pool(name="ps_et", bufs=2, space="PSUM") as ps_et,
            tc.tile_pool(name="ps_enc", bufs=2, space="PSUM") as ps_enc,
            tc.tile_pool(name="ps_asum", bufs=2, space="PSUM") as ps_asum,
        ):
            # ---- constants: two packed blobs, one DMA each ----
            # cblob32 cols: [0:512] cw (rows 0:32), [512:544] dsb,
            #              [544] ebias (rows 0:32), [545:577] i32 (rows 0:32)
            # cblob16 cols: [0:128] w1 (4 c-chunks x 32), [128] ones
            cb32 = consts.tile([128, 577], f32)
            nc.gpsimd.dma_start(out=cb32, in_=cb32_d[:])
            cb16 = consts.tile([128, 161], bf16)
            nc.gpsimd.dma_start(out=cb16, in_=cb16_d[:])
            cw_sb = cb32[0:K, 0:512]
            dsb = cb32[:, 512:544]
            eb_sb = cb32[0:K, 544:545]
            i32_sb = cb32[0:K, 545:577]
            on_sb = cb16[:, 128:129]
            i32bf = cb16[0:K, 129:161]

            # Loop fission: phase-1 of batch b+1 is emitted before phase-2 of
            # batch b so the in-order PE stream has fill work while batch b's
            # normalize chain runs on DVE/ACT.
            st = [{} for _ in range(BPC)]
            # natural-layout loads for BOTH batches issue first on the sync
            # ring so neither batch's phase-1 waits behind transpose issue
            for b in range(BPC):
                xn_sb = xn_pool.tile([128, CC, N], bf16)
                for cc in range(CC):
                    nc.sync.dma_start(
                        out=xn_sb[:, cc, :],
                        in_=xn_d[b, cc * 128:(cc + 1) * 128, :],
                    )
                st[b]["xn_sb"] = xn_sb
            for b in range(BPC):
                xn_sb = st[b]["xn_sb"]
                xt_sb = xt_pool.tile([128, NCHUNKS, C], bf16)
                x2mat = x2m_pool.tile([128, NCHUNKS], f32)
                # separate scratches so DVE- and ACT-side squares don't
                # serialize on a shared WAW chain
                sqd = sq_pool.tile([128, C], bf16)
                sqa = sq_pool.tile([128, C], bf16)
                for g in range(NCHUNKS // 4):
                    # one xbar DMA transposes 4 chunks: the 3D out AP folds
                    # the mid-dim into logical partitions p-inner, so
                    # out[:, q, :] lands as natural n-chunk 4g+q
                    nc.sync.dma_start_transpose(
                        out=xt_sb[:, 4 * g:4 * g + 4, :],
                        in_=xn_d[b, :, 512 * g:512 * (g + 1)],
                    )
                for ch in range(NCHUNKS):
                    # x2[n] via fused square + free-dim sum, split
                    # DVE (scalar_tensor_tensor) / ACT (Square + accum_out)
                    if ch % 8 < 5:
                        nc.vector.scalar_tensor_tensor(
                            out=sqd,
                            in0=xt_sb[:, ch, :],
                            scalar=1.0,
                            in1=xt_sb[:, ch, :],
                            op0=ALU.mult,
                            op1=ALU.mult,
                            accum_out=x2mat[:, ch:ch + 1],
                        )
                    else:
                        nc.scalar.activation(
                            out=sqa,
                            in_=xt_sb[:, ch, :],
                            func=AF.Square,
                            accum_out=x2mat[:, ch:ch + 1],
                        )

                # recenter: x2c = x2 - 256 > 0 (f32, feeds the exp-factor path)
                x2c = x2m_pool.tile([128, NCHUNKS], f32)
                nc.vector.tensor_scalar_add(out=x2c, in0=x2mat, scalar1=-X2C)

                # ---- phase 1 + softmax numerator, per 512-slice ----
                eall = eall_pool.tile([128, NCHUNKS, K], f32)
                et = ps_et.tile([128, NCHUNKS, K], bf16)
                for s in range(NSLICES):
                    S = ps_s.tile([K, 512], f32)
                    for cc in range(CC):
                        nc.tensor.matmul(
                            S,
                            lhsT=cb16[:, 32 * cc:32 * (cc + 1)],
                            rhs=xn_sb[:, cc, s * 512:(s + 1) * 512],
                            start=(cc == 0),
                            stop=(cc == CC - 1),
                        )
                    e_sb = e_pool.tile([K, 512], bf16)
                    nc.scalar.activation(
                        out=e_sb, in_=S, func=AF.Exp, bias=eb_sb, scale=1.0
                    )
                    for q in range(4):
                        ch = 4 * s + q
                        nc.tensor.transpose(
                            out=et[:, ch, :],
                            in_=e_sb[:, q * 128:(q + 1) * 128],
                            identity=i32bf,
                        )
                st[b].update(xt_sb=xt_sb, x2c=x2c, eall=eall, et=et)

            for b in range(BPC):
                xt_sb = st[b]["xt_sb"]
                x2c = st[b]["x2c"]
                eall = st[b]["eall"]
                et = st[b]["et"]
                # ---- x2 factor: e = e' * exp(ds_k * x2c[n]), then normalize
                # (per-chunk ts_mul keeps einsum2's in-order PE MMs startable
                # chunk-by-chunk) ----
                F = f_pool.tile([128, NCHUNKS, K], f32)
                nc.vector.tensor_mul(
                    F,
                    bass.AP(tensor=x2c.tensor, offset=x2c.offset,
                            ap=[x2c.ap[0], x2c.ap[1], [0, K]]),
                    bass.AP(tensor=dsb.tensor, offset=dsb.offset,
                            ap=[dsb.ap[0], [0, NCHUNKS], dsb.ap[1]]),
                )
                eF = f_pool.tile([128, NCHUNKS, K], f32)
                nc.scalar.activation(out=eF, in_=F, func=AF.Exp)
                nc.vector.tensor_mul(eall, et, eF)
                zmat = z_pool.tile([128, NCHUNKS], f32)
                nc.vector.reduce_sum(out=zmat, in_=eall, axis=mybir.AxisListType.X)
                rz = z_pool.tile([128, NCHUNKS], f32)
                nc.vector.reciprocal(out=rz, in_=zmat)
                a_sb = a_pool.tile([128, NCHUNKS, K], bf16)
                for ch in range(NCHUNKS):
                    nc.vector.tensor_scalar_mul(
                        out=a_sb[:, ch, :],
                        in0=eall[:, ch, :],
                        scalar1=rz[:, ch:ch + 1],
                    )

                # ---- phase 2: enc1 = A^T @ xT, asum = A^T @ 1 ----
                enc_ps = ps_enc.tile([K, C], f32)
                asum_ps = ps_asum.tile([K, 1], f32)
                for ch in range(NCHUNKS):
                    nc.tensor.matmul(
                        enc_ps,
                        lhsT=a_sb[:, ch, :],
                        rhs=xt_sb[:, ch, :],
                        start=(ch == 0),
                        stop=(ch == NCHUNKS - 1),
                    )
                    nc.tensor.matmul(
                        asum_ps,
                        lhsT=a_sb[:, ch, :],
                        rhs=on_sb,
                        start=(ch == 0),
                        stop=(ch == NCHUNKS - 1),
                    )
                nasum = nasum_pool.tile([K, 1], f32)
                nc.scalar.activation(
                    out=nasum, in_=asum_ps, func=AF.Copy, bias=0.0, scale=-1.0
                )
                enc_sb = enc_sb_pool.tile([K, C], f32)
                nc.vector.scalar_tensor_tensor(
                    out=enc_sb,
                    in0=cw_sb,
                    scalar=nasum,
                    in1=enc_ps,
                    op0=ALU.mult,
                    op1=ALU.add,
                )
                nc.sync.dma_start(out=enc_d[b], in_=enc_sb)

    if not nc.is_finalized():
        nc.finalize()
    return nc


def _host_prep(x, codewords, scale):
    bf = ml_dtypes.bfloat16
    xf = np.ascontiguousarray(x.reshape(B, C, N)).astype(bf)
    s64 = scale.astype(np.float64)
    cw64 = codewords.astype(np.float64)
    smax = s64.max()
    ds64 = s64 - smax                                   # [K]
    w1 = (-2.0 * s64[:, None] * cw64).T                 # [C, K]
    w1 = np.ascontiguousarray(w1.reshape(CC, 128, K)).astype(bf)
    c2 = (cw64 * cw64).sum(axis=1)                      # [K]
    ebias = (s64 * c2 + ds64 * X2C).astype(np.float32).reshape(K, 1)
    cb32 = np.zeros((128, 577), dtype=np.float32)
    cb32[0:K, 0:512] = codewords.astype(np.float32)
    cb32[:, 512:544] = ds64.astype(np.float32).reshape(1, K)
    cb32[0:K, 544:545] = ebias
    cb32[0:K, 545:577] = np.eye(K, dtype=np.float32)
    cb16 = np.zeros((128, 161), dtype=bf)
    for cc in range(CC):
        cb16[:, 32 * cc:32 * (cc + 1)] = w1[cc]
    cb16[:, 128] = 1.0
    cb16[0:K, 129:161] = np.eye(K, dtype=np.float32)
    consts = {"cblob32": cb32, "cblob16": cb16}
    return xf, consts


def kernel(x, codewords, scale, _trace=False):
    from concourse.bass_utils import run_bass_kernel_spmd

    if "nc" not in _cache:
        _cache["nc"] = _build_nc()
    nc = _cache["nc"]

    xf, consts = _host_prep(
        np.asarray(x), np.asarray(codewords), np.asarray(scale)
    )
    in_maps = []
    for i in range(NCORES):
        m = dict(consts)
        m["xn"] = np.ascontiguousarray(xf[i * BPC:(i + 1) * BPC])
        in_maps.append(m)

    res = run_bass_kernel_spmd(
        nc, in_maps, list(range(NCORES)), trace=_trace
    )
    out = np.empty((B, K, C), dtype=np.float32)
    for i in range(NCORES):
        out[i * BPC:(i + 1) * BPC] = res.results[i]["enc"]
    if _trace:
        _cache["last_exec_time_ns"] = res.exec_time_ns
    return out


# revision 50
# speedup vs baseline: 1.0608x; 1.0608x over previous
"""VQ codebook encoding kernel for Trainium2 (8 NeuronCores, SPMD).

Problem: nn_Encoding-style soft-assignment codebook encoding.
  x: (16, 512, 64, 64) f32, codewords: (32, 512) f32, scale: (32,) f32
  logits[b,n,k] = scale[k] * (||x_bn||^2 - 2 x_bn.c_k + ||c_k||^2)
  A = softmax_k(logits);  out[b,k,c] = sum_n A (x_bn - c_k)   -> (16, 32, 512)

Sharding: data-parallel over batch B=16 -> 2 batches per core, no collectives.

Per-core dataflow (matmul operands bf16, accumulation/softmax f32):
  - x shard is cast to bf16 on host; loaded twice from HBM: natural layout
    [c,n] (contiguous per c-chunk) and transposed [n,c] via the xbar
    DMA-transpose path (one DMA per 4 n-chunks; 3D out AP folds the mid-dim
    into logical partitions in natural chunk order).
  - phase 1 (PE): S'[k,n] = sum_c W1[c,k] x[c,n] in PSUM, where
    W1 = -2*scale_k*cw[k,c]; exp on ACT with per-partition bias
    s_k*c2_k + ds_k*X2C (ds = scale - scale.max()) -> e'; PE-transpose
    e' -> [n-part, k] PSUM.
  - x2[n] = sum_c xT^2 via fused square+row-sum ops split across DVE
    (scalar_tensor_tensor accum_out) and ACT (Square accum_out).
  - softmax shift is exact for any shared per-n shift: the x2 term is applied
    AFTER the transpose as e = e' * exp(ds_k * (x2[n]-X2C)) (one broadcast
    tensor_mul + one ACT exp + one tensor_mul); ds<=0 and x2>X2C keep the
    factor in (0,1], and e' flushes only where the true weight is negligible.
  - Z = sum_k e (DVE row-reduce), reciprocal, normalize -> A (bf16).
  - phase 2 (PE): enc1[k,c] = sum_n A^T xT, asum[k] = sum_n A via ones
    column; out = enc1 - asum*cw fused on DVE (scalar_tensor_tensor); DMA out.
  - Loop fission: both batches' phase-1 emitted before phase-2s so the
    in-order PE stream stays fed during the DVE/ACT normalize chains.
"""

import numpy as np
import ml_dtypes

B, C, H, W = 16, 512, 64, 64
K = 32
N = H * W            # 4096 spatial positions
NCORES = 8
BPC = B // NCORES    # batches per core
CC = C // 128        # c chunks (4)
NSLICES = N // 512   # 8 matmul slices per batch
NCHUNKS = N // 128   # 32 n-chunks per batch
X2C = 256.0          # x2 recentering: ds<=0 and x2-256>0 keep exp(ds*(x2-X2C)) in (0,1]

_cache = {}


def _build_nc():
    import concourse.bass as bass
    import concourse.bacc as bacc
    import concourse.tile as tile
    from concourse import mybir

    f32 = mybir.dt.float32
    bf16 = mybir.dt.bfloat16
    AF = mybir.ActivationFunctionType
    ALU = mybir.AluOpType

    # Bacc (not plain Bass): its compile pipeline splits semaphore waits to
    # the 1-per-instruction hardware limit and codegens ISA subclasses —
    # required for this walrus build to accept the NEFF.
    nc = bacc.Bacc("TRN2", target_bir_lowering=False, debug=False)

    xn_d = nc.declare_dram_parameter("xn", [BPC, C, N], bf16, isOutput=False)
    cb32_d = nc.declare_dram_parameter("cblob32", [128, 577], f32, isOutput=False)
    cb16_d = nc.declare_dram_parameter("cblob16", [128, 161], bf16, isOutput=False)
    enc_d = nc.declare_dram_parameter("enc", [BPC, K, C], f32, isOutput=True)

    with tile.TileContext(nc) as tc:
        with (
            tc.tile_pool(name="consts", bufs=1) as consts,
            tc.tile_pool(name="xn", bufs=2) as xn_pool,
            tc.tile_pool(name="xt", bufs=2) as xt_pool,
            tc.tile_pool(name="sq", bufs=2) as sq_pool,
            tc.tile_pool(name="x2m", bufs=2) as x2m_pool,
            tc.tile_pool(name="fmat", bufs=2) as f_pool,
            tc.tile_pool(name="e", bufs=3) as e_pool,
            tc.tile_pool(name="eall", bufs=2) as eall_pool,
            tc.tile_pool(name="z", bufs=2) as z_pool,
            tc.tile_pool(name="a", bufs=2) as a_pool,
            tc.tile_pool(name="encsb", bufs=2) as enc_sb_pool,
            tc.tile_pool(name="nasum", bufs=2) as nasum_pool,
            tc.tile_pool(name="ps_s", bufs=2, space="PSUM") as ps_s,
            tc.tile_pool(name="ps_et", bufs=2, space="PSUM") as ps_et,
            tc.tile_pool(name="ps_enc", bufs=2, space="PSUM") as ps_enc,
            tc.tile_pool(name="ps_asum", bufs=2, space="PSUM") as ps_asum,
        ):
            # ---- constants: two packed blobs, one DMA each ----
            # cblob32 cols: [0:512] cw (rows 0:32), [512:544] dsb,
            #              [544] ebias (rows 0:32), [545:577] i32 (rows 0:32)
            # cblob16 cols: [0:128] w1 (4 c-chunks x 32), [128] ones
            cb32 = consts.tile([128, 577], f32)
            nc.gpsimd.dma_start(out=cb32, in_=cb32_d[:])
            cb16 = consts.tile([128, 161], bf16)
            nc.gpsimd.dma_start(out=cb16, in_=cb16_d[:])
            cw_sb = cb32[0:K, 0:512]
            dsb = cb32[:, 512:544]
            eb_sb = cb32[0:K, 544:545]
            i32_sb = cb32[0:K, 545:577]
            on_sb = cb16[:, 128:129]
            i32bf = cb16[0:K, 129:161]

            # Loop fission: phase-1 of batch b+1 is emitted before phase-2 of
            # batch b so the in-order PE stream has fill work while batch b's
            # normalize chain runs on DVE/ACT.
            st = [{} for _ in range(BPC)]
            # natural-layout loads for BOTH batches issue first on the sync
            # ring so neither batch's phase-1 waits behind transpose issue
            for b in range(BPC):
                xn_sb = xn_pool.tile([128, CC, N], bf16)
                for cc in range(CC):
                    nc.sync.dma_start(
                        out=xn_sb[:, cc, :],
                        in_=xn_d[b, cc * 128:(cc + 1) * 128, :],
                    )
                st[b]["xn_sb"] = xn_sb
            for b in range(BPC):
                xn_sb = st[b]["xn_sb"]
                xt_sb = xt_pool.tile([128, NCHUNKS, C], bf16)
                x2mat = x2m_pool.tile([128, NCHUNKS], f32)
                # separate scratches so DVE- and ACT-side squares don't
                # serialize on a shared WAW chain
                sqd = sq_pool.tile([128, C], bf16)
                sqa = sq_pool.tile([128, C], bf16)
                for g in range(NCHUNKS // 4):
                    # one xbar DMA transposes 4 chunks: the 3D out AP folds
                    # the mid-dim into logical partitions p-inner, so
                    # out[:, q, :] lands as natural n-chunk 4g+q
                    nc.sync.dma_start_transpose(
                        out=xt_sb[:, 4 * g:4 * g + 4, :],
                        in_=xn_d[b, :, 512 * g:512 * (g + 1)],
                    )
                for ch in range(NCHUNKS):
                    # x2[n] via fused square + free-dim sum, split
                    # DVE (scalar_tensor_tensor) / ACT (Square + accum_out)
                    if ch % 16 < 9:
                        nc.vector.scalar_tensor_tensor(
                            out=sqd,
                            in0=xt_sb[:, ch, :],
                            scalar=1.0,
                            in1=xt_sb[:, ch, :],
                            op0=ALU.mult,
                            op1=ALU.mult,
                            accum_out=x2mat[:, ch:ch + 1],
                        )
                    else:
                        nc.scalar.activation(
                            out=sqa,
                            in_=xt_sb[:, ch, :],
                            func=AF.Square,
                            accum_out=x2mat[:, ch:ch + 1],
                        )

                # recenter: x2c = x2 - 256 > 0 (f32, feeds the exp-factor path)
                x2c = x2m_pool.tile([128, NCHUNKS], f32)
                nc.vector.tensor_scalar_add(out=x2c, in0=x2mat, scalar1=-X2C)

                # ---- phase 1 + softmax numerator, per 512-slice ----
                eall = eall_pool.tile([128, NCHUNKS, K], f32)
                et = ps_et.tile([128, NCHUNKS, K], bf16)
                for s in range(NSLICES):
                    S = ps_s.tile([K, 512], f32)
                    for cc in range(CC):
                        nc.tensor.matmul(
                            S,
                            lhsT=cb16[:, 32 * cc:32 * (cc + 1)],
                            rhs=xn_sb[:, cc, s * 512:(s + 1) * 512],
                            start=(cc == 0),
                            stop=(cc == CC - 1),
                        )
                    e_sb = e_pool.tile([K, 512], bf16)
                    nc.scalar.activation(
                        out=e_sb, in_=S, func=AF.Exp, bias=eb_sb, scale=1.0
                    )
                    for q in range(4):
                        ch = 4 * s + q
                        nc.tensor.transpose(
                            out=et[:, ch, :],
                            in_=e_sb[:, q * 128:(q + 1) * 128],
                            identity=i32bf,
                        )
                st[b].update(xt_sb=xt_sb, x2c=x2c, eall=eall, et=et)

            for b in range(BPC):
                xt_sb = st[b]["xt_sb"]
                x2c = st[b]["x2c"]
                eall = st[b]["eall"]
                et = st[b]["et"]
                # ---- x2 factor: e = e' * exp(ds_k * x2c[n]), then normalize
                # (per-chunk ts_mul keeps einsum2's in-order PE MMs startable
                # chunk-by-chunk) ----
                F = f_pool.tile([128, NCHUNKS, K], f32)
                nc.vector.tensor_mul(
                    F,
                    bass.AP(tensor=x2c.tensor, offset=x2c.offset,
                            ap=[x2c.ap[0], x2c.ap[1], [0, K]]),
                    bass.AP(tensor=dsb.tensor, offset=dsb.offset,
                            ap=[dsb.ap[0], [0, NCHUNKS], dsb.ap[1]]),
                )
                eF = f_pool.tile([128, NCHUNKS, K], f32)
                nc.scalar.activation(out=eF, in_=F, func=AF.Exp)
                nc.vector.tensor_mul(eall, et, eF)
                zmat = z_pool.tile([128, NCHUNKS], f32)
                nc.vector.reduce_sum(out=zmat, in_=eall, axis=mybir.AxisListType.X)
                rz = z_pool.tile([128, NCHUNKS], f32)
                nc.vector.reciprocal(out=rz, in_=zmat)
                a_sb = a_pool.tile([128, NCHUNKS, K], bf16)
                for ch in range(NCHUNKS):
                    nc.vector.tensor_scalar_mul(
                        out=a_sb[:, ch, :],
                        in0=eall[:, ch, :],
                        scalar1=rz[:, ch:ch + 1],
                    )

                # ---- phase 2: enc1 = A^T @ xT, asum = A^T @ 1 ----
                enc_ps = ps_enc.tile([K, C], f32)
                asum_ps = ps_asum.tile([K, 1], f32)
                for ch in range(NCHUNKS):
                    nc.tensor.matmul(
                        enc_ps,
                        lhsT=a_sb[:, ch, :],
                        rhs=xt_sb[:, ch, :],
                        start=(ch == 0),
                        stop=(ch == NCHUNKS - 1),
                    )
                    nc.tensor.matmul(
                        asum_ps,
                        lhsT=a_sb[:, ch, :],
                        rhs=on_sb,
                        start=(ch == 0),
                        stop=(ch == NCHUNKS - 1),
                    )
                nasum = nasum_pool.tile([K, 1], f32)
                nc.scalar.activation(
                    out=nasum, in_=asum_ps, func=AF.Copy, bias=0.0, scale=-1.0
                )
                enc_sb = enc_sb_pool.tile([K, C], f32)
                nc.vector.scalar_tensor_tensor(
                    out=enc_sb,
                    in0=cw_sb,
                    scalar=nasum,
                    in1=enc_ps,
                    op0=ALU.mult,
                    op1=ALU.add,
                )
                nc.sync.dma_start(out=enc_d[b], in_=enc_sb)

    if not nc.is_finalized():
        nc.finalize()
    return nc


def _host_prep(x, codewords, scale):
    bf = ml_dtypes.bfloat16
    xf = np.ascontiguousarray(x.reshape(B, C, N)).astype(bf)
    s64 = scale.astype(np.float64)
    cw64 = codewords.astype(np.float64)
    smax = s64.max()
    ds64 = s64 - smax                                   # [K]
    w1 = (-2.0 * s64[:, None] * cw64).T                 # [C, K]
    w1 = np.ascontiguousarray(w1.reshape(CC, 128, K)).astype(bf)
    c2 = (cw64 * cw64).sum(axis=1)                      # [K]
    ebias = (s64 * c2 + ds64 * X2C).astype(np.float32).reshape(K, 1)
    cb32 = np.zeros((128, 577), dtype=np.float32)
    cb32[0:K, 0:512] = codewords.astype(np.float32)
    cb32[:, 512:544] = ds64.astype(np.float32).reshape(1, K)
    cb32[0:K, 544:545] = ebias
    cb32[0:K, 545:577] = np.eye(K, dtype=np.float32)
    cb16 = np.zeros((128, 161), dtype=bf)
    for cc in range(CC):
        cb16[:, 32 * cc:32 * (cc + 1)] = w1[cc]
    cb16[:, 128] = 1.0
    cb16[0:K, 129:161] = np.eye(K, dtype=np.float32)
    consts = {"cblob32": cb32, "cblob16": cb16}
    return xf, consts


def kernel(x, codewords, scale, _trace=False):
    from concourse.bass_utils import run_bass_kernel_spmd

    if "nc" not in _cache:
        _cache["nc"] = _build_nc()
    nc = _cache["nc"]

    xf, consts = _host_prep(
        np.asarray(x), np.asarray(codewords), np.asarray(scale)
    )
    in_maps = []
    for i in range(NCORES):
        m = dict(consts)
        m["xn"] = np.ascontiguousarray(xf[i * BPC:(i + 1) * BPC])
        in_maps.append(m)

    res = run_bass_kernel_spmd(
        nc, in_maps, list(range(NCORES)), trace=_trace
    )
    out = np.empty((B, K, C), dtype=np.float32)
    for i in range(NCORES):
        out[i * BPC:(i + 1) * BPC] = res.results[i]["enc"]
    if _trace:
        _cache["last_exec_time_ns"] = res.exec_time_ns
    return out
